# revision 1
# baseline (speedup 1.0000x reference)
"""Trainium2 Bass kernel for Dynamic ReLU-B (nn_Dynamic_Relu_B_70291434766473).

Reference computation (per sample n, channel c, pixel p):
    pooled[n,c] = mean_p x[n,c,p]
    h = relu(pooled @ fc1_w.T + fc1_b)                       # [N, 32]
    delta = 2*sigmoid(einsum('koh,nh->kno', fc2_w, h) + fc2_b) - 1
    alpha = delta[..., 0::2]; beta = delta[..., 1::2]        # [K, N, C]
    a = [1,0][k] + 1.0*alpha ; b = [1,0][k] + 0.5*beta
    out = max_k (x * a[k] + b[k])

Strategy: pure data parallel over batch N=32 across 8 NeuronCores (4
samples/core).  Per core the x-shard (12.8 MB) stays SBUF-resident.

  - x[n] loads as two [128, 3136] channel-half tiles; ch0 rides the SP
    HWDGE ring (nc.sync), ch1 the ACT HWDGE ring (nc.scalar); the two
    rings share the ~430 GB/s SBUF-AXI fabric.  Stores ride the same
    ring as their channel half, behind the loads in FIFO order.  Small
    constants load via the SWDGE (gpsimd) queues so they don't block.
    The first two samples load in pixel-halves so pooling starts
    during the DMA ramp.
  - pooling via accum_out (free-dim sum; the 1/HW normalizer is folded
    into the fc1 weights host-side): ScalarE activation(Copy) for ch0,
    and for the early samples VectorE tensor_scalar pools ch1 in its
    otherwise-idle ramp window, halving the pool latency.
  - per-sample MLP: fc1 = K-split fp32 matmuls accumulating the pooled
    partials in PSUM; fc2 = 2 wide matmuls with the [33, 1024] weight
    matrix as the *moving* operand and the tiny [33, 1] h-vector
    stationary (fc2 bias via a ones-row on h); 2*sigmoid(z)-1 is
    evaluated as tanh(z/2) in one activation, then PE-transposed in
    [1, 128] chunks to land per-channel params on partitions.
  - apply: branch k=0 on VectorE tensor_scalar (x*a0+b0, 2x mode),
    branch k=1 on ScalarE activation(Identity, scale=a1, bias=b1),
    max on VectorE tensor_tensor; the last samples run in pixel-halves
    to shorten the tail before the end-of-kernel barrier.
  - emission order interleaves pools/MLPs/applies so each engine's
    compile-time queue order matches real data-arrival order.
"""

import numpy as np

N, C, H, W = 32, 256, 56, 56
HW = H * W
HID = C // 8  # 32
NCORES = 8
NPC = N // NCORES  # samples per core
BATCH = 1          # samples per MLP batch

_CACHE = {}


def _build_program():
    """Build (and cache) the compiled Bass program for one core."""
    if "nc" in _CACHE:
        return _CACHE["nc"]

    import concourse.bacc as bacc
    import concourse.mybir as mybir
    import concourse.tile as tile

    f32 = mybir.dt.float32
    AF = mybir.ActivationFunctionType
    ALU = mybir.AluOpType

    nc = bacc.Bacc(
        "TRN2",
        target_bir_lowering=False,
        debug=False,
        enable_asserts=True,
        num_devices=NCORES,
    )

    xs = nc.dram_tensor("xs", [NPC, C, HW], f32, kind="ExternalInput").ap()
    w1t = nc.dram_tensor("w1t", [C, HID], f32, kind="ExternalInput").ap()
    fc1b = nc.dram_tensor("fc1b", [HID, 1], f32, kind="ExternalInput").ap()
    w2r = nc.dram_tensor("w2r", [HID + 1, 8 * 128], f32, kind="ExternalInput").ap()
    ident = nc.dram_tensor("ident", [BATCH, BATCH], f32, kind="ExternalInput").ap()
    out = nc.dram_tensor("out", [NPC, C, HW], f32, kind="ExternalOutput").ap()

    ring = {0: nc.sync, 1: nc.scalar}  # per-channel-half HWDGE ring

    with tile.TileContext(nc) as tc:
        with (
            tc.tile_pool(name="const", bufs=1) as cpool,
            tc.tile_pool(name="x", bufs=2 * NPC) as xpool,
            tc.tile_pool(name="y1", bufs=3) as ypool,
            tc.tile_pool(name="o", bufs=4) as opool,
            tc.tile_pool(name="th", bufs=2) as thpool,
            tc.tile_pool(name="small", bufs=1) as smpool,
            tc.tile_pool(name="ps", bufs=2, space="PSUM") as pspool,
        ):
            # --- constants (SWDGE queues; don't block the HWDGE rings) ---
            w1t_t = []
            for ch in range(2):
                t = cpool.tile([128, HID], f32, tag=f"w1t{ch}")
                nc.gpsimd.dma_start(t[:], w1t[ch * 128:(ch + 1) * 128, :])
                w1t_t.append(t)
            fc1b_t = cpool.tile([HID, 1], f32, tag="fc1b")
            nc.gpsimd.dma_start(fc1b_t[:], fc1b[:])
            w2r_t = cpool.tile([HID + 1, 8 * 128], f32, tag="w2r")
            nc.gpsimd.dma_start(w2r_t[:], w2r[:])
            id_t = cpool.tile([BATCH, BATCH], f32, tag="ident")
            nc.gpsimd.dma_start(id_t[:], ident[:])

            # --- load all x tiles (ch0 -> sync ring, ch1 -> scalar ring);
            # the first two samples stream in pixel-halves so pooling can
            # start earlier during the DMA ramp ---
            HH = HW // 2
            SPLIT_LOAD = (0, 1)   # samples loaded (and pooled) in halves
            DVE_POOL = (0, 1)     # samples whose ch1 pools run on VectorE
            SPLIT_APPLY = (2, 3)  # samples whose apply runs in halves
            DVE_Y1 = {(2, 0)}     # y1 branches computed on VectorE instead
            ACT_Y0 = set()        # y0 stays on VectorE (parallel with y1)
            GPSIMD_MAX = set()  # walrus rejects 2-input TT on Pool
            xt = {}
            for n in range(NPC):
                for ch in range(2):
                    t = xpool.tile([128, HW], f32, tag="x")
                    if n in SPLIT_LOAD:
                        for h in range(2):
                            ring[ch].dma_start(
                                t[:, h * HH:(h + 1) * HH],
                                xs[n, ch * 128:(ch + 1) * 128,
                                   h * HH:(h + 1) * HH],
                            )
                    else:
                        ring[ch].dma_start(
                            t[:], xs[n, ch * 128:(ch + 1) * 128, :]
                        )
                    xt[(n, ch)] = t

            pl, tts, abs_, tanh_insts = {}, {}, {}, {}
            B2, B4 = 2 * BATCH, 4 * BATCH

            def pool_sample(n):
                # accum_out = sum over pixels -> [128, 1] per part; the
                # full-size dump target borrows a y1 slot.  Split-loaded
                # samples pool each pixel-half as soon as it lands; fc1
                # accumulates the partial sums in PSUM.
                halves = (
                    [slice(0, HH), slice(HH, HW)]
                    if n in SPLIT_LOAD else [slice(0, HW)]
                )
                scr0 = ypool.tile([128, HW], f32, tag="y1")
                scr1 = ypool.tile([128, HW], f32, tag="y1")
                scr = {0: scr0, 1: scr1}
                pl[(n, 0)], pl[(n, 1)] = [], []
                # emit in (half, ch) order: the two rings deliver ch0/ch1
                # in parallel, so this matches real arrival order and keeps
                # ScalarE from idling on the not-yet-landed second half.
                # For the early samples VectorE is still idle, so it pools
                # the ch1 tiles in parallel with ScalarE pooling ch0.
                for h, sl in enumerate(halves):
                    for ch in range(2):
                        p = smpool.tile([128, 1], f32, tag=f"pl{n}{ch}{h}")
                        if ch == 1 and n in DVE_POOL:
                            nc.vector.tensor_scalar(
                                scr[ch][:, sl], xt[(n, ch)][:, sl], 1.0,
                                None, ALU.mult, ALU.add, accum_out=p[:],
                            )
                        else:
                            nc.scalar.activation(
                                scr[ch][:, sl], xt[(n, ch)][:, sl],
                                AF.Copy, accum_out=p[:],
                            )
                        pl[(n, ch)].append(p)

            def mlp_batch(b):
                ss = range(b * BATCH, (b + 1) * BATCH)
                # fc1 per sample: ph = (fc1_w/HW) @ xsum, then relu+bias
                ht = smpool.tile([HID + 1, BATCH], f32, tag=f"h{b}")
                nc.scalar.activation(  # ones row for the fc2 bias trick
                    ht[HID:HID + 1, :], w1t_t[0][0:1, 0:BATCH],
                    AF.Copy, bias=1.0, scale=0.0,
                )
                for s in ss:
                    ph = pspool.tile([HID, 1], f32, tag="ph")
                    terms = [
                        (ch, p) for ch in range(2) for p in pl[(s, ch)]
                    ]
                    for ti, (ch, p) in enumerate(terms):
                        nc.tensor.matmul(
                            ph[:], w1t_t[ch][:], p[:],
                            start=(ti == 0), stop=(ti == len(terms) - 1),
                        )
                    nc.scalar.activation(
                        ht[0:HID, s - b * BATCH:s - b * BATCH + 1], ph[:],
                        AF.Relu, bias=fc1b_t[:], scale=1.0,
                    )
                # fc2: z.T = ht.T @ w2r -> [BATCH, 1024], bias via ones row
                pz = pspool.tile([BATCH, 8 * 128], f32, tag="pz")
                for k in range(2):
                    nc.tensor.matmul(
                        pz[:, k * 512:(k + 1) * 512],
                        ht[:], w2r_t[:, k * 512:(k + 1) * 512],
                        start=True, stop=True,
                    )
                # t = tanh((z+b2)/2) = 2*sigmoid(z+b2) - 1
                th = thpool.tile([BATCH, 8 * 128], f32, tag="th")
                tanh_insts[b] = nc.scalar.activation(
                    th[:], pz[:], AF.Tanh, bias=0.0, scale=0.5
                )
                # transpose [BATCH, 128] chunks -> [128, BATCH] each
                tp = pspool.tile([128, 8 * BATCH], f32, tag="tp")
                for j in range(8):
                    nc.tensor.transpose(
                        tp[:, j * BATCH:(j + 1) * BATCH],
                        th[:, j * 128:(j + 1) * 128], id_t[:],
                    )
                tt = smpool.tile([128, 8 * BATCH], f32, tag=f"tt{b}")
                nc.vector.tensor_copy(tt[:], tp[:])
                # cols of tt: j*BATCH + i, j = k*4 + isbeta*2 + ch, i = s-2b
                #   a0 = 1 + t     b0 = 0.5*t + 1
                #   a1 = t (straight from tt)    b1 = 0.5*t
                ab = smpool.tile([128, 8 * BATCH], f32, tag=f"ab{b}")
                nc.vector.tensor_scalar_add(ab[:, 0:B2], tt[:, 0:B2], 1.0)
                nc.vector.tensor_scalar(
                    ab[:, B2:B4], tt[:, B2:B4], 0.5, 1.0, ALU.mult, ALU.add
                )
                nc.vector.tensor_scalar_mul(
                    ab[:, 3 * B2:4 * B2], tt[:, 3 * B2:4 * B2], 0.5
                )
                tts[b], abs_[b] = tt, ab

            def apply_batch(b, chs=(0, 1), act_after=None):
                # emit all y1s, then all y0s, then the maxes, so neither
                # engine's queue head-of-line blocks on the other engine.
                # SPLIT_APPLY samples run in pixel-halves (shorter tail).
                tt, ab = tts[b], abs_[b]
                units = []  # (s, ch, q, pixel-slice)
                for s in range(b * BATCH, (b + 1) * BATCH):
                    for ch in chs:
                        q = ch * BATCH + (s - b * BATCH)
                        if s in SPLIT_APPLY:
                            units.append((s, ch, q, slice(0, HH)))
                            units.append((s, ch, q, slice(HH, HW)))
                        else:
                            units.append((s, ch, q, slice(0, HW)))
                y1s, os_ = {}, {}
                for u, (s, ch, q, sl) in enumerate(units):
                    y1 = ypool.tile([128, sl.stop - sl.start], f32, tag="y1")
                    if (s, ch) in DVE_Y1:
                        nc.vector.tensor_scalar(
                            y1[:], xt[(s, ch)][:, sl],
                            tt[:, 2 * B2 + q:2 * B2 + q + 1],
                            ab[:, 3 * B2 + q:3 * B2 + q + 1],
                            ALU.mult, ALU.add,
                        )
                    else:
                        inst = nc.scalar.activation(
                            y1[:], xt[(s, ch)][:, sl], AF.Identity,
                            bias=ab[:, 3 * B2 + q:3 * B2 + q + 1],
                            scale=tt[:, 2 * B2 + q:2 * B2 + q + 1],
                        )
                        if act_after is not None and u >= 1:
                            # keep the next batch's tanh (critical tail
                            # chain) ahead of these streaming ops on ACT
                            tile.add_dep_helper(
                                inst.ins, act_after.ins, sync=False,
                                reason="tail tanh before late y1s",
                            )
                    y1s[u] = y1
                for u, (s, ch, q, sl) in enumerate(units):
                    o = opool.tile([128, sl.stop - sl.start], f32, tag="o")
                    if (s, ch) in ACT_Y0:
                        nc.scalar.activation(
                            o[:], xt[(s, ch)][:, sl], AF.Identity,
                            bias=ab[:, B2 + q:B2 + q + 1],
                            scale=ab[:, q:q + 1],
                        )
                    else:
                        nc.vector.tensor_scalar(
                            o[:], xt[(s, ch)][:, sl],
                            ab[:, q:q + 1], ab[:, B2 + q:B2 + q + 1],
                            ALU.mult, ALU.add,
                        )
                    os_[u] = o
                for u, (s, ch, q, sl) in enumerate(units):
                    o, y1 = os_[u], y1s[u]
                    nc.vector.tensor_max(o[:], o[:], y1[:])
                    if s == NPC - 1:
                        # final sample: split each store across both rings
                        # so the tail transfers drain in parallel
                        m = (sl.stop - sl.start) // 2
                        ring[0].dma_start(
                            out[s, ch * 128:(ch + 1) * 128,
                                sl.start:sl.start + m], o[:, 0:m],
                        )
                        ring[1].dma_start(
                            out[s, ch * 128:(ch + 1) * 128,
                                sl.start + m:sl.stop], o[:, m:],
                        )
                    else:
                        ring[ch].dma_start(
                            out[s, ch * 128:(ch + 1) * 128, sl], o[:]
                        )

            # pools lead (they pace on DMA arrival), each sample's MLP as
            # soon as it is pooled, applies stream behind
            pool_sample(0)
            mlp_batch(0)
            pool_sample(1)
            apply_batch(0)
            mlp_batch(1)
            pool_sample(2)
            mlp_batch(2)
            apply_batch(1)
            pool_sample(3)
            mlp_batch(3)
            apply_batch(2)
            apply_batch(3)

    nc.compile()
    _CACHE["nc"] = nc
    return nc


def make_inputs(x, fc1_w, fc1_b, fc2_w, fc2_b):
    """Host-side prep: shard x, rearrange weights into device layouts."""
    x = np.ascontiguousarray(x, dtype=np.float32).reshape(N, C, HW)
    # fc1: transpose + fold the 1/HW pooling normalizer into the weights
    w1t = np.ascontiguousarray(fc1_w.T.astype(np.float32) / np.float32(HW))
    fc1b = np.ascontiguousarray(fc1_b.astype(np.float32).reshape(HID, 1))
    # fc2 as the *moving* matmul operand: [HID+1, 1024] with col o=j*128+c,
    # j = k*4 + isbeta*2 + ch; row HID carries fc2_b (ones-row trick)
    w2r = np.zeros((HID + 1, 8 * 128), np.float32)
    for k in range(2):
        for isbeta in range(2):
            wab = fc2_w[k, isbeta::2, :].astype(np.float32)  # [256, 32]
            bab = fc2_b[k, isbeta::2].astype(np.float32)     # [256]
            for ch in range(2):
                j = k * 4 + isbeta * 2 + ch
                sl = slice(j * 128, (j + 1) * 128)
                w2r[:HID, sl] = wab[128 * ch:128 * (ch + 1), :].T
                w2r[HID, sl] = bab[128 * ch:128 * (ch + 1)]
    ident = np.eye(BATCH, dtype=np.float32)
    in_maps = []
    for i in range(NCORES):
        in_maps.append({
            "xs": np.ascontiguousarray(x[NPC * i:NPC * (i + 1)]),
            "w1t": w1t,
            "fc1b": fc1b,
            "w2r": w2r,
            "ident": ident,
        })
    return in_maps


def kernel(x, fc1_w, fc1_b, fc2_w, fc2_b):
    from concourse.bass_utils import run_bass_kernel_spmd

    nc = _build_program()
    in_maps = make_inputs(x, fc1_w, fc1_b, fc2_w, fc2_b)
    res = run_bass_kernel_spmd(nc, in_maps, core_ids=list(range(NCORES)))
    shards = [res.results[i]["out"] for i in range(NCORES)]
    return np.concatenate(shards, axis=0).reshape(N, C, H, W)


if __name__ == "__main__":
    rng = np.random.default_rng(0)
    x = rng.standard_normal((N, C, H, W), dtype=np.float32)
    fc1_w = rng.standard_normal((HID, C), dtype=np.float32) * 0.06
    fc1_b = rng.standard_normal((HID,), dtype=np.float32) * 0.06
    fc2_w = rng.standard_normal((2, 2 * C, HID), dtype=np.float32) * 0.17
    fc2_b = rng.standard_normal((2, 2 * C), dtype=np.float32) * 0.17
    out = kernel(x, fc1_w, fc1_b, fc2_w, fc2_b)
    print(out.shape, out.dtype)



# revision 5
# speedup vs baseline: 1.5317x; 1.5317x over previous
"""Trainium2 Bass kernel for Dynamic ReLU-B (nn_Dynamic_Relu_B_70291434766473).

Reference computation (per sample n, channel c, pixel p):
    pooled[n,c] = mean_p x[n,c,p]
    h = relu(pooled @ fc1_w.T + fc1_b)                       # [N, 32]
    delta = 2*sigmoid(einsum('koh,nh->kno', fc2_w, h) + fc2_b) - 1
    alpha = delta[..., 0::2]; beta = delta[..., 1::2]        # [K, N, C]
    a = [1,0][k] + 1.0*alpha ; b = [1,0][k] + 0.5*beta
    out = max_k (x * a[k] + b[k])

Strategy: pure data parallel over batch N=32 across 8 NeuronCores (4
samples/core).  x and out move through HBM as bf16 (host casts), which
halves DMA traffic to ~12.9 MB/core and unlocks the DVE 16-bit perf
modes (4x tensor_scalar, 2x tensor_tensor).  End-to-end bf16 error is
~5e-3 vs the 2e-2 gate.

  - pooling runs on the otherwise-idle PE via linearity:
    fc1_pre = W1 @ (sum_p x) = sum_p (W1 @ x).  Per sample, 8 pixel
    chunks of 392 go through matmuls with the [128, 32] fc1 weights,
    landing in 4 partition-bands x 2 column-halves of one [128, 784]
    PSUM tile; a single accumulate pass per column-half (one on ACT,
    one on DVE) gives band partials, and a tiny [128, 32] summing
    selector matmul folds the 4 bands and lands fc1-preact on
    partitions 0..31.  relu(scale=1/HW, bias=fc1b) -> ht column.
  - fc2 is computed transposed: the [33, 128] w2r chunks are the
    stationary operand and the tiny ht column the moving one, so
    z lands per-channel on partitions directly ([128, 8] PSUM);
    2*sigmoid(z)-1 = tanh(z/2) is one [128, 8] ACT op.  No PE
    transposes needed.
  - apply: branch k=1 on ScalarE activation(Identity, scale=a1,
    bias=b1) in pixel-halves (keeps the ACT queue responsive for the
    tiny head ops), late tiles on VectorE; branch k=0 and the max on
    VectorE (bf16 4x / 2x modes); params stay fp32 per-partition
    scalars (exempt from the 16-bit perf-mode dtype rule).
  - DMA: x loads ride ch0->SP ring, ch1->ACT ring (parallel ramp),
    all stores ride the SP ring behind the ch0 loads; the last
    sample's stores split across both rings.  Constants go via the
    SWDGE (gpsimd) queues.  First two samples load in pixel-halves,
    last two samples apply in pixel-halves (shorter ramp/tail).
"""

import numpy as np

N, C, H, W = 32, 256, 56, 56
HW = H * W
HID = C // 8  # 32
NCORES = 8
NPC = N // NCORES  # samples per core
NCH = 8            # pixel chunks for the PE pooling matmuls
CHK = HW // NCH    # 392

_CACHE = {}


def _build_program():
    """Build (and cache) the compiled Bass program for one core."""
    if "nc" in _CACHE:
        return _CACHE["nc"]

    import concourse.bacc as bacc
    import concourse.mybir as mybir
    import concourse.tile as tile

    f32 = mybir.dt.float32
    bf16 = mybir.dt.bfloat16
    AF = mybir.ActivationFunctionType
    ALU = mybir.AluOpType

    nc = bacc.Bacc(
        "TRN2",
        target_bir_lowering=False,
        debug=False,
        enable_asserts=True,
        num_devices=NCORES,
    )

    xs = nc.dram_tensor("xs", [NPC, C, HW], bf16, kind="ExternalInput").ap()
    w1tp = nc.dram_tensor("w1tp", [2, 128, HID], bf16, kind="ExternalInput").ap()
    sump = nc.dram_tensor("sump", [128, HID], f32, kind="ExternalInput").ap()
    fc1b = nc.dram_tensor("fc1b", [HID, 1], f32, kind="ExternalInput").ap()
    w2r = nc.dram_tensor("w2r", [HID + 1, 8 * 128], bf16, kind="ExternalInput").ap()
    out = nc.dram_tensor("out", [NPC, C, HW], bf16, kind="ExternalOutput").ap()

    ring = {0: nc.sync, 1: nc.scalar}  # per-channel-half HWDGE ring

    HH = HW // 2
    SPLIT_LOAD = (0, 1)   # samples loaded in pixel-halves
    SPLIT_APPLY = (2, 3)  # samples whose apply runs in pixel-halves
    # y1 tiles on ACT (rest on DVE); late samples lean DVE for the tail
    ACT_Y1 = {(0, 0), (0, 1), (1, 0), (1, 1), (2, 0)}

    with tile.TileContext(nc) as tc:
        with (
            tc.tile_pool(name="const", bufs=1) as cpool,
            tc.tile_pool(name="x", bufs=2 * NPC) as xpool,
            tc.tile_pool(name="y1", bufs=3) as ypool,
            tc.tile_pool(name="o", bufs=4) as opool,
            tc.tile_pool(name="rd", bufs=2) as rpool,
            tc.tile_pool(name="small", bufs=1) as smpool,
            tc.tile_pool(name="ps", bufs=2, space="PSUM") as pspool,
        ):
            # --- constants (SWDGE queues; don't block the HWDGE rings) ---
            w1tp_t = []
            for ch in range(2):
                t = cpool.tile([128, HID], bf16, tag=f"w1tp{ch}")
                nc.gpsimd.dma_start(t[:], w1tp[ch])
                w1tp_t.append(t)
            sump_t = cpool.tile([128, HID], f32, tag="sump")
            nc.gpsimd.dma_start(sump_t[:], sump[:])
            fc1b_t = cpool.tile([HID, 1], f32, tag="fc1b")
            nc.gpsimd.dma_start(fc1b_t[:], fc1b[:])
            w2r_t = cpool.tile([HID + 1, 8 * 128], bf16, tag="w2r")
            nc.gpsimd.dma_start(w2r_t[:], w2r[:])

            # ht holds all samples' hidden vectors; row HID is the ones row
            # for the fc2 bias trick (set once via the ACT copy trick).
            ht = smpool.tile([HID + 1, NPC], bf16, tag="ht")
            nc.scalar.activation(
                ht[HID:HID + 1, :], w2r_t[HID:HID + 1, 0:NPC],
                AF.Copy, bias=1.0, scale=0.0,
            )

            # --- load all x tiles (ch0 -> sync ring, ch1 -> scalar ring) ---
            xt = {}
            for n in range(NPC):
                for ch in range(2):
                    t = xpool.tile([128, HW], bf16, tag="x")
                    if n in SPLIT_LOAD:
                        for h in range(2):
                            ring[ch].dma_start(
                                t[:, h * HH:(h + 1) * HH],
                                xs[n, ch * 128:(ch + 1) * 128,
                                   h * HH:(h + 1) * HH],
                            )
                    else:
                        ring[ch].dma_start(
                            t[:], xs[n, ch * 128:(ch + 1) * 128, :]
                        )
                    xt[(n, ch)] = t

            tts, abs_ = {}, {}

            def head(s):
                # G = W1 @ x via PE: chunk j -> band j%4, col-half j//4.
                # [128, 1024] = exactly 2 PSUM banks; col-half 1 starts at
                # col 512 so no matmul crosses a bank boundary.
                g = pspool.tile([128, 1024], f32, tag="g")
                for j in range(NCH):
                    band = j % 4
                    col = (j // 4) * 512
                    for ch in range(2):
                        nc.tensor.matmul(
                            g[32 * band:32 * band + 32, col:col + CHK],
                            w1tp_t[ch][:],
                            xt[(s, ch)][:, j * CHK:(j + 1) * CHK],
                            start=(ch == 0), stop=(ch == 1),
                            tile_position=(0, 32 * band),
                        )
                # reduce each column-half over pixels -> band partials
                pre2 = smpool.tile([128, 2], f32, tag=f"pre{s}")
                dA = rpool.tile([128, CHK], f32, tag="dA")
                dB = rpool.tile([128, CHK], f32, tag="dB")
                nc.scalar.activation(
                    dA[:], g[:, 0:CHK], AF.Copy, accum_out=pre2[:, 0:1],
                )
                nc.vector.tensor_scalar(
                    dB[:], g[:, 512:512 + CHK], 1.0, None,
                    ALU.mult, ALU.add, accum_out=pre2[:, 1:2],
                )
                # band-sum + partition shift onto 0..31 via selector matmul
                php = pspool.tile([HID, 1], f32, tag="php")
                nc.tensor.matmul(
                    php[:], sump_t[:], pre2[:, 0:1], start=True, stop=False,
                )
                nc.tensor.matmul(
                    php[:], sump_t[:], pre2[:, 1:2], start=False, stop=True,
                )
                # fc1: relu(php/HW + fc1b) -> ht column s
                nc.scalar.activation(
                    ht[0:HID, s:s + 1], php[:],
                    AF.Relu, bias=fc1b_t[:], scale=1.0 / HW,
                )
                # fc2 transposed: z lands per-channel on partitions
                pzt = pspool.tile([128, 8], f32, tag="pzt")
                for cb in range(8):
                    nc.tensor.matmul(
                        pzt[:, cb:cb + 1],
                        w2r_t[:, cb * 128:(cb + 1) * 128],
                        ht[:, s:s + 1],
                        start=True, stop=True,
                    )
                # t = tanh(z/2) = 2*sigmoid(z) - 1
                tt = smpool.tile([128, 8], f32, tag=f"tt{s}")
                nc.scalar.activation(tt[:], pzt[:], AF.Tanh, bias=0.0, scale=0.5)
                tts[s] = tt

            def ab_ops(s):
                # cols j = k*4 + isbeta*2 + ch:
                #   a0 = 1 + t   b0 = 0.5*t + 1   a1 = t   b1 = 0.5*t
                tt = tts[s]
                ab = smpool.tile([128, 8], f32, tag=f"ab{s}")
                nc.vector.tensor_scalar_add(ab[:, 0:2], tt[:, 0:2], 1.0)
                nc.vector.tensor_scalar(
                    ab[:, 2:4], tt[:, 2:4], 0.5, 1.0, ALU.mult, ALU.add
                )
                nc.vector.tensor_scalar_mul(ab[:, 6:8], tt[:, 6:8], 0.5)
                abs_[s] = ab

            def apply_s(s):
                # emit all y1s, then all y0s, then the maxes, so neither
                # engine's queue head-of-line blocks on the other engine.
                tt, ab = tts[s], abs_[s]
                units = []  # (ch, pixel-slice) at store granularity
                for ch in range(2):
                    if s in SPLIT_APPLY:
                        units.append((ch, slice(0, HH)))
                        units.append((ch, slice(HH, HW)))
                    else:
                        units.append((ch, slice(0, HW)))
                y1s, os_ = {}, {}
                for u, (ch, sl) in enumerate(units):
                    y1 = ypool.tile([128, sl.stop - sl.start], bf16, tag="y1")
                    if (s, ch) in ACT_Y1:
                        # halves keep the ACT queue responsive for head ops
                        m = (sl.stop - sl.start) // 2
                        for hs in (slice(0, m), slice(m, sl.stop - sl.start)):
                            nc.scalar.activation(
                                y1[:, hs],
                                xt[(s, ch)][:, sl.start + hs.start:
                                            sl.start + hs.stop],
                                AF.Identity,
                                bias=ab[:, 6 + ch:7 + ch],
                                scale=tt[:, 4 + ch:5 + ch],
                            )
                    else:
                        nc.vector.tensor_scalar(
                            y1[:], xt[(s, ch)][:, sl],
                            tt[:, 4 + ch:5 + ch], ab[:, 6 + ch:7 + ch],
                            ALU.mult, ALU.add,
                        )
                    y1s[u] = y1
                for u, (ch, sl) in enumerate(units):
                    o = opool.tile([128, sl.stop - sl.start], bf16, tag="o")
                    nc.vector.tensor_scalar(
                        o[:], xt[(s, ch)][:, sl],
                        ab[:, ch:ch + 1], ab[:, 2 + ch:3 + ch],
                        ALU.mult, ALU.add,
                    )
                    os_[u] = o
                for u, (ch, sl) in enumerate(units):
                    o, y1 = os_[u], y1s[u]
                    nc.vector.tensor_max(o[:], o[:], y1[:])
                    if s == NPC - 1:
                        # final sample: split each store across both rings
                        # so the tail transfers drain in parallel
                        m = (sl.stop - sl.start) // 2
                        ring[0].dma_start(
                            out[s, ch * 128:(ch + 1) * 128,
                                sl.start:sl.start + m], o[:, 0:m],
                        )
                        ring[1].dma_start(
                            out[s, ch * 128:(ch + 1) * 128,
                                sl.start + m:sl.stop], o[:, m:],
                        )
                    else:
                        ring[0].dma_start(
                            out[s, ch * 128:(ch + 1) * 128, sl], o[:]
                        )

            # heads lead (they pace on DMA arrival); each sample's apply
            # streams behind; ab ops sit right before their apply so the
            # DVE queue never head-of-line blocks on a later tanh.
            head(0)
            head(1)
            ab_ops(0)
            apply_s(0)
            head(2)
            ab_ops(1)
            apply_s(1)
            head(3)
            ab_ops(2)
            apply_s(2)
            ab_ops(3)
            apply_s(3)

    nc.compile()
    _CACHE["nc"] = nc
    return nc


def make_inputs(x, fc1_w, fc1_b, fc2_w, fc2_b):
    """Host-side prep: shard x, rearrange weights into device layouts."""
    import ml_dtypes

    bf16 = ml_dtypes.bfloat16
    x = np.ascontiguousarray(
        np.asarray(x, dtype=np.float32).reshape(N, C, HW)
    ).astype(bf16)
    # fc1 weights transposed, per channel-half (1/HW folded into the relu)
    w1tp = np.ascontiguousarray(
        fc1_w.T.astype(np.float32).reshape(2, 128, HID)
    ).astype(bf16)
    # band-summing selector: SumP[32*m + h, h] = 1
    sump = np.ascontiguousarray(
        np.tile(np.eye(HID, dtype=np.float32), (4, 1))
    )
    fc1b = np.ascontiguousarray(fc1_b.astype(np.float32).reshape(HID, 1))
    # fc2 as [HID+1, 1024] with col o = j*128 + c, j = k*4 + isbeta*2 + ch;
    # row HID carries fc2_b (ones-row trick)
    w2r = np.zeros((HID + 1, 8 * 128), np.float32)
    for k in range(2):
        for isbeta in range(2):
            wab = fc2_w[k, isbeta::2, :].astype(np.float32)  # [256, 32]
            bab = fc2_b[k, isbeta::2].astype(np.float32)     # [256]
            for ch in range(2):
                j = k * 4 + isbeta * 2 + ch
                sl = slice(j * 128, (j + 1) * 128)
                w2r[:HID, sl] = wab[128 * ch:128 * (ch + 1), :].T
                w2r[HID, sl] = bab[128 * ch:128 * (ch + 1)]
    w2r = w2r.astype(bf16)
    in_maps = []
    for i in range(NCORES):
        in_maps.append({
            "xs": np.ascontiguousarray(x[NPC * i:NPC * (i + 1)]),
            "w1tp": w1tp,
            "sump": sump,
            "fc1b": fc1b,
            "w2r": w2r,
        })
    return in_maps


def kernel(x, fc1_w, fc1_b, fc2_w, fc2_b):
    from concourse.bass_utils import run_bass_kernel_spmd

    nc = _build_program()
    in_maps = make_inputs(x, fc1_w, fc1_b, fc2_w, fc2_b)
    res = run_bass_kernel_spmd(nc, in_maps, core_ids=list(range(NCORES)))
    shards = [
        np.asarray(res.results[i]["out"]).astype(np.float32)
        for i in range(NCORES)
    ]
    return np.concatenate(shards, axis=0).reshape(N, C, H, W)


if __name__ == "__main__":
    rng = np.random.default_rng(0)
    x = rng.standard_normal((N, C, H, W), dtype=np.float32)
    fc1_w = rng.standard_normal((HID, C), dtype=np.float32) * 0.06
    fc1_b = rng.standard_normal((HID,), dtype=np.float32) * 0.06
    fc2_w = rng.standard_normal((2, 2 * C, HID), dtype=np.float32) * 0.17
    fc2_b = rng.standard_normal((2, 2 * C), dtype=np.float32) * 0.17
    out = kernel(x, fc1_w, fc1_b, fc2_w, fc2_b)
    print(out.shape, out.dtype)
